# revision 1
# baseline (speedup 1.0000x reference)
"""Trainium2 Bass kernel for nn_CumulativeFlattenedLinear (segment_reduce).

Computation: per window of S=64 timesteps, per-timestep C->O linear projection
(weights zero for the first n_discard steps) followed by a causal cumsum within
the window, plus bias.

Strategy (data-parallel over batch, 1 batch element per core):
  - Reformulate per 8-step sub-block u: a triangular-masked "intra" matmul plus
    a "prefix" matmul whose target axis is the later sub-blocks; both share a
    transposed-x stationary and are issued as ONE stacked N=256 fp32r matmul
    writing [intra | pre] contiguously in PSUM (pre region shared per u-pair,
    accumulated in PSUM).
  - x is loaded with partition = 256-element time chunk (1KB contiguous DMA
    runs), shuffled on-chip to (u, c, v') column order (GPSIMD), transposed
    128x128 on the TensorEngine, rounded to fp32r during the batched
    PSUM->SBUF copies (ScalarE).
  - prefix totals summed across the 3 pair-regions + bias (DVE), then one
    strided combine per window writes the (o, t)-ordered output tile, stored
    with 1KB contiguous runs.
"""
import numpy as np

import concourse.bass as bass
import concourse.tile as tile
from concourse import bacc, mybir
from concourse.bass_utils import run_bass_kernel_spmd

F32 = mybir.dt.float32
F32R = mybir.dt.float32r

# problem geometry (asserted against inputs at runtime)
B, C, T, O = 8, 16, 131072, 16
P = 128
CH = 256                 # time-elements per partition per supertile
NST = T // (P * CH)      # 4 supertiles
V = 8                    # sub-block length
NU = 8                   # sub-blocks per window

_cache = {}


def _build_nc(du_count, mm_dtype=F32R):
    """Build the per-core Bass program. du_count = number of active sub-blocks
    (those with any nonzero weight), assumed to be the trailing ones."""
    S = NU * V  # 64
    NW = CH // S  # windows per partition = 4
    DU = du_count
    first_u = NU - DU          # first active sub-block
    fill_s = first_u * V       # s < fill_s -> output = bias

    nc = bacc.Bacc("TRN2", target_bir_lowering=False, debug=False)
    x_d = nc.dram_tensor("x", (C, T), F32, kind="ExternalInput")
    w_d = nc.dram_tensor("w_all", (P, DU * 256), mm_dtype, kind="ExternalInput")
    bpre_d = nc.dram_tensor("bias_pre", (P, P), F32, kind="ExternalInput")
    ident_d = nc.dram_tensor("ident", (P, P), F32, kind="ExternalInput")
    bfill_d = nc.dram_tensor("bias_fill", (P, O * fill_s), F32,
                             kind="ExternalInput")
    y_d = nc.dram_tensor("y", (O, T), F32, kind="ExternalOutput")

    xv = x_d.ap().rearrange("c (st p hs) -> st p c hs", st=NST, p=P, hs=CH)
    yv = y_d.ap().rearrange("o (st p hs) -> st p o hs", st=NST, p=P, hs=CH)

    NB = (DU + 1) // 2  # psum banks per window group

    with tile.TileContext(nc) as tc:
        with (
            tc.tile_pool(name="const", bufs=1) as cp,
            tc.tile_pool(name="io", bufs=2) as io,
            tc.tile_pool(name="mid", bufs=3) as mid,
            tc.tile_pool(name="psW", bufs=2, space="PSUM") as psW,
            tc.tile_pool(name="psT", bufs=2, space="PSUM") as psT,
        ):
            w_all = cp.tile([P, DU * 256], mm_dtype, name="w_all")
            nc.sync.dma_start(w_all[:], w_d.ap())
            bias_pre = cp.tile([P, P], F32, name="bias_pre")
            nc.sync.dma_start(bias_pre[:], bpre_d.ap())
            ident = cp.tile([P, P], F32, name="ident")
            nc.sync.dma_start(ident[:], ident_d.ap())
            bfill = cp.tile([P, O * fill_s], F32, name="bfill")
            nc.sync.dma_start(bfill[:], bfill_d.ap())

            for st in range(NST):
                xin = io.tile([P, C * CH], F32, name="xin", tag="xin")
                nc.sync.dma_start(
                    xin[:].rearrange("p (c hs) -> p c hs", c=C), xv[st]
                )
                out_sb = io.tile([P, O * CH], F32, name="out_sb", tag="out")
                for wdw in range(NW):
                    # ---- shuffle to (du, c, v') ----
                    shuf = mid.tile([P, DU * 128], F32, name="shuf", tag="shuf")
                    src = xin[:].rearrange(
                        "p (c w u v) -> w p u c v", c=C, w=NW, u=NU, v=V
                    )[wdw, :, first_u:NU]
                    nc.vector.tensor_copy(
                        shuf[:].rearrange("p (u c v) -> p u c v", u=DU, c=C, v=V),
                        src,
                    )
                    # ---- transposes (PE) in groups of <=4 per psum bank ----
                    tsb = []
                    du = 0
                    gi = 0
                    while du < DU:
                        n = min(4, DU - du)
                        pt = psT.tile([P, 512], F32, name=f"pt{gi}", tag="pt")
                        for j in range(n):
                            nc.tensor.transpose(
                                pt[:, j * 128:(j + 1) * 128],
                                shuf[:, (du + j) * 128:(du + j + 1) * 128],
                                ident[:],
                                tile_position=(0, 0),
                            )
                        ts = mid.tile([P, n * 128], mm_dtype,
                                      name=f"ts{gi}", tag=f"ts{gi}")
                        nc.scalar.copy(ts[:], pt[:, 0:n * 128])
                        for j in range(n):
                            tsb.append((ts, j))
                        du += n
                        gi += 1
                    # ---- matmuls ----
                    pw = psW.tile([P, NB * 512], F32, name="pw", tag="pw")
                    for du in range(DU):
                        bk = du // 2
                        lo = bk * 512 + (du % 2) * 128
                        ts, j = tsb[du]
                        nc.tensor.matmul(
                            pw[:, lo:lo + 256],
                            ts[:, j * 128:(j + 1) * 128],
                            w_all[:, du * 256:(du + 1) * 256],
                            start=(du % 2 == 0),
                            stop=(du % 2 == 1 or du == DU - 1),
                            skip_group_check=True,
                        )
                    # ---- prefix totals: pre_s = sum of pre regions ----
                    pre_s = mid.tile([P, P], F32, name="pre_s", tag="pre_s")
                    nc.vector.tensor_add(pre_s[:], bias_pre[:], pw[:, 128:256])
                    for bk in range(1, NB):
                        nc.vector.tensor_add(
                            pre_s[:], pre_s[:],
                            pw[:, bk * 512 + 128:bk * 512 + 256],
                        )
                    # ---- combine: out[(o, s)] = intra + pre_bcast ----
                    # out col = o*CH + wdw*S + s,  s = (first_u + du)*V + v
                    out4 = out_sb[:].rearrange(
                        "p (o w u v) -> w p o u v", o=O, w=NW, u=NU, v=V
                    )[wdw, :, :, first_u:NU]
                    # in1: psum intra: col = bk*512 + (du%2)*256 + v*16 + o
                    in1 = pw[:].rearrange(
                        "p (bk half x) -> p bk half x", bk=NB, half=2
                    )[:, :, :, 0:128]
                    in1 = in1.rearrange(
                        "p bk half (v o) -> p o (bk half) v", v=V, o=O
                    )
                    # in2: pre_s col = (first_u + du)*16 + o, step0 over v
                    in2 = pre_s[:].rearrange("p (u o) -> p u o", u=NU)
                    in2 = in2[:, first_u:NU]
                    in2 = in2.transpose([0, 2, 1]).unsqueeze(3)
                    in2 = in2.broadcast_to([P, O, DU, V])
                    nc.vector.tensor_add(out4, in1, in2)
                    # ---- bias fill for s < fill_s (ACT) ----
                    outf = out_sb[:].rearrange(
                        "p (o w s) -> w p o s", o=O, w=NW
                    )[wdw, :, :, 0:fill_s]
                    nc.scalar.copy(
                        outf,
                        bfill[:].rearrange("p (o s) -> p o s", o=O),
                    )
                nc.scalar.dma_start(
                    yv[st], out_sb[:].rearrange("p (o hs) -> p o hs", o=O)
                )
    nc.compile()
    return nc


def _host_constants(weight, bias, n_discard, n_keep, mm_np=np.float32):
    S = n_discard + n_keep
    assert S == NU * V
    w = weight.reshape(O, C, n_keep).transpose(2, 1, 0)  # (n_keep, C, O)
    w_full = np.concatenate(
        [np.zeros((n_discard, C, O), np.float32), w.astype(np.float32)], axis=0
    )  # (S, C, O)
    act = [u for u in range(NU)
           if np.abs(w_full[u * V:(u + 1) * V]).max() > 0]
    # kernel assumes active blocks are trailing & contiguous
    first_u = act[0] if act else NU
    assert act == list(range(first_u, NU))
    DU = len(act)
    rhs = np.zeros((DU, P, 256), np.float32)
    vp_idx = np.arange(V)
    for idx, u in enumerate(act):
        blk = w_full[u * V:(u + 1) * V]  # (V, C, O)
        # Wtri: k=(c,vp) -> n=(v,o)
        tri = np.zeros((C, V, V, O), np.float32)
        for v in range(V):
            tri[:, vp_idx <= v, v, :] = blk.transpose(1, 0, 2)[:, vp_idx <= v]
        Wtri = tri.reshape(C * V, V * O)
        # Wpre: k=(c,vp) -> n=(ut,o)
        pre = np.zeros((C, V, NU, O), np.float32)
        for ut in range(NU):
            if ut > u:
                pre[:, :, ut, :] = blk.transpose(1, 0, 2)
        Wpre = pre.reshape(C * V, NU * O)
        if idx % 2 == 0:
            rhs[idx] = np.concatenate([Wtri, Wpre], axis=1)
        else:
            rhs[idx] = np.concatenate([Wpre, Wtri], axis=1)
    w_all = rhs.transpose(1, 0, 2).reshape(P, DU * 256).astype(mm_np)
    bias32 = bias.astype(np.float32)
    consts = {
        "w_all": np.ascontiguousarray(w_all),
        "bias_pre": np.ascontiguousarray(
            np.tile(bias32, NU)[None, :] * np.ones((P, 1), np.float32)
        ),
        "ident": np.eye(P, dtype=np.float32),
        "bias_fill": np.ascontiguousarray(
            np.tile(bias32[:, None], (1, first_u * V)).reshape(1, -1)
            * np.ones((P, 1), np.float32)
        ),
    }
    return consts, DU


def _run(inputs, trace=False):
    x = np.asarray(inputs["x"], dtype=np.float32)
    weight = np.asarray(inputs["weight"], dtype=np.float32)
    bias = np.asarray(inputs["bias"], dtype=np.float32)
    n_discard = int(inputs["n_discard"])
    n_keep = int(inputs["n_keep"])
    assert x.shape == (B, C, T) and weight.shape == (O, C * n_keep)

    consts, DU = _host_constants(weight, bias, n_discard, n_keep)
    key = ("nc", DU)
    if key not in _cache:
        _cache[key] = _build_nc(DU)
    nc = _cache[key]

    in_maps = []
    for b in range(B):
        m = dict(consts)
        m["x"] = np.ascontiguousarray(x[b])
        in_maps.append(m)
    res = run_bass_kernel_spmd(nc, in_maps, list(range(B)), trace=trace)
    y = np.stack([res.results[b]["y"] for b in range(B)], axis=0)
    return y, res


def kernel(**inputs):
    y, _ = _run(inputs, trace=False)
    return y



# revision 8
# speedup vs baseline: 1.2547x; 1.2547x over previous
"""Trainium2 Bass kernel for nn_CumulativeFlattenedLinear (segment_reduce).

Per window of S=64 timesteps: per-timestep C->O projection (weights zero for
the first n_discard steps) + causal cumsum within the window, plus bias.

Strategy (data-parallel over batch, 1 batch element per core), fp16 I/O:
  - x and y cross HBM as fp16 (host converts) with 1KB contiguous runs:
    partition = 512-element time chunk, 2 supertiles. Loads/stores are split
    across the two HWDGE queues (SP + ACT) to overlap both directions.
  - Per window: 6 fp16 PE transposes build x^T (k=(c,v')) in PSUM, one ACT
    copy moves it to SBUF; 6 triangular "intra" matmuls write the window's
    (s,o)-major PSUM region; block totals accumulate into a single shared
    128-col PSUM "pre" region seeded with bias by a K=1 matmul, so prefix
    sums come out of PSUM accumulation for free.
  - One DVE broadcast-add per window evicts intra+prefix to the fp16 output
    tile; GPSIMD fills the leading n_discard positions with bias.
"""
import numpy as np

import concourse.bass as bass
import concourse.tile as tile
from concourse import bacc, mybir
from concourse.bass_utils import run_bass_kernel_spmd

F16 = mybir.dt.float16
F32 = mybir.dt.float32

B, C, T, O = 8, 16, 131072, 16
P = 128
CH = 512                  # time elems per partition per supertile (1KB fp16)
NST = T // (P * CH)       # 2 supertiles
V = 8                     # sub-block length
NU = 8                    # sub-blocks per window
S = NU * V                # 64
NW = CH // S              # windows per partition row = 8

_cache = {}

# eviction reads prefix totals straight from PSUM (2 PSUM operands on DVE);
# flip to True if hardware rejects that and route through an SBUF copy
PRE_VIA_SBUF = True


def _build_nc(first_u):
    DU = NU - first_u          # active sub-blocks (trailing)
    NPRE = DU - 1              # blocks contributing prefix totals
    fill_s = first_u * V       # s < fill_s -> output = bias

    nc = bacc.Bacc("TRN2", target_bir_lowering=False, debug=False)
    x_d = nc.dram_tensor("x", (C, T), F16, kind="ExternalInput")
    wi_d = nc.dram_tensor("w_intra", (P, DU * 128), F16, kind="ExternalInput")
    wp_d = nc.dram_tensor("w_pre", (P, max(NPRE, 1) * 128), F16,
                          kind="ExternalInput")
    ident_d = nc.dram_tensor("ident", (P, P), F16, kind="ExternalInput")
    ones_d = nc.dram_tensor("ones_k1", (1, P), F16, kind="ExternalInput")
    brow_d = nc.dram_tensor("biasrow", (1, P), F16, kind="ExternalInput")
    bfill_d = nc.dram_tensor("bias_fill", (P, max(O * fill_s, 1)), F16,
                             kind="ExternalInput")
    y_d = nc.dram_tensor("y", (O, T), F16, kind="ExternalOutput")

    xv = x_d.ap().rearrange("c (st p hs) -> st p c hs", st=NST, p=P, hs=CH)
    yv = y_d.ap().rearrange("o (st p hs) -> st p o hs", st=NST, p=P, hs=CH)

    MMBASE = 2 * P             # mm region starts at col 256 (s>=16)
    PREBASE = S * O            # pre region at col 1024..1152

    with tile.TileContext(nc) as tc:
        with (
            tc.tile_pool(name="const", bufs=1) as cp,
            tc.tile_pool(name="io", bufs=2) as io,
            tc.tile_pool(name="shf", bufs=3) as shf,
            tc.tile_pool(name="tsb", bufs=3) as tsb,
            tc.tile_pool(name="mid", bufs=3) as mid,
            tc.tile_pool(name="psT", bufs=2, space="PSUM") as psT,
            tc.tile_pool(name="psW", bufs=2, space="PSUM") as psW,
        ):
            w_intra = cp.tile([P, DU * 128], F16, name="w_intra")
            nc.sync.dma_start(w_intra[:], wi_d.ap())
            w_pre = cp.tile([P, max(NPRE, 1) * 128], F16, name="w_pre")
            nc.sync.dma_start(w_pre[:], wp_d.ap())
            ident = cp.tile([P, P], F16, name="ident")
            nc.sync.dma_start(ident[:], ident_d.ap())
            ones = cp.tile([1, P], F16, name="ones_k1")
            nc.sync.dma_start(ones[:], ones_d.ap())
            brow = cp.tile([1, P], F16, name="biasrow")
            nc.sync.dma_start(brow[:], brow_d.ap())
            bfill = cp.tile([P, max(O * fill_s, 1)], F16, name="bfill")
            nc.sync.dma_start(bfill[:], bfill_d.ap())

            # all input loads issued up-front, split across both HWDGE queues
            xins = []
            for st in range(NST):
                xin = io.tile([P, C * CH], F16, name=f"xin{st}", tag="xin")
                xr = xin[:].rearrange("p (c hs) -> p c hs", c=C)
                nc.sync.dma_start(xr[0:64], xv[st, 0:64])
                nc.scalar.dma_start(xr[64:128], xv[st, 64:128])
                xins.append(xin)
            outs = [io.tile([P, O * CH], F16, name=f"out{st}", tag="out")
                    for st in range(NST)]

            state = {}

            def front(st, w, widx):
                xin = xins[st]
                # shuffle window columns to (u, c, v) blocks; alternate the
                # engine so neither DVE nor GPSIMD becomes the bottleneck
                sh = shf.tile([P, DU * 128], F16, name="shuf", tag="shuf")
                src = xin[:].rearrange(
                    "p (c w u v) -> w p u c v", c=C, w=NW, u=NU, v=V
                )[w][:, first_u:NU]
                eng = nc.vector if widx % 2 == 0 else nc.gpsimd
                eng.tensor_copy(
                    sh[:].rearrange("p (u c v) -> p u c v", u=DU, c=C, v=V),
                    src,
                )
                pt = psT.tile([P, DU * 128], F16, name="pt", tag="pt")
                for du in range(DU):
                    nc.tensor.transpose(
                        pt[:, du * 128:(du + 1) * 128],
                        sh[:, du * 128:(du + 1) * 128],
                        ident[:],
                    )
                ts = tsb.tile([P, DU * 128], F16, name="ts", tag="ts")
                nc.scalar.copy(ts[:], pt[:])
                state[(st, w)] = ts

            def back(st, w):
                ts = state.pop((st, w))
                out_sb = outs[st]
                pw = psW.tile([P, PREBASE + P], F32, name="pw", tag="pw")
                # seed pre region with bias (sets has_written for accumulate)
                nc.tensor.matmul(
                    pw[:, PREBASE:PREBASE + P], ones[:], brow[:],
                    start=True, stop=(NPRE == 0), skip_group_check=True,
                )
                for du in range(DU):
                    lo = MMBASE + du * 128
                    nc.tensor.matmul(
                        pw[:, lo:lo + 128],
                        ts[:, du * 128:(du + 1) * 128],
                        w_intra[:, du * 128:(du + 1) * 128],
                        start=True, stop=True, skip_group_check=True,
                    )
                for pu in range(NPRE):
                    nc.tensor.matmul(
                        pw[:, PREBASE:PREBASE + P],
                        ts[:, pu * 128:(pu + 1) * 128],
                        w_pre[:, pu * 128:(pu + 1) * 128],
                        start=False, stop=(pu == NPRE - 1),
                        skip_group_check=True,
                    )
                if PRE_VIA_SBUF:
                    pre_sb = mid.tile([P, P], F32, name="pre_sb", tag="pre_sb")
                    nc.scalar.copy(pre_sb[:], pw[:, PREBASE:PREBASE + P])
                    pre_ap = pre_sb[:, first_u * O:]
                else:
                    pre_ap = pw[:, PREBASE + first_u * O:PREBASE + P]
                # eviction: out[(o, s)] = intra + prefix, fp16
                out4 = out_sb[:].rearrange(
                    "p (o w u v) -> w p o u v", o=O, w=NW, u=NU, v=V
                )[w][:, :, first_u:NU]
                in1 = pw[:, MMBASE:MMBASE + DU * 128].rearrange(
                    "p (u v o) -> p o u v", u=DU, v=V, o=O
                )
                in2 = pre_ap.rearrange(
                    "p (u o) -> p o u", u=DU, o=O
                ).unsqueeze(3).broadcast_to([P, O, DU, V])
                nc.vector.tensor_add(out4, in1, in2)
                # bias fill for s < fill_s (GPSIMD)
                if fill_s:
                    outf = out_sb[:].rearrange(
                        "p (o w s) -> w p o s", o=O, w=NW
                    )[w][:, :, 0:fill_s]
                    nc.gpsimd.tensor_copy(
                        outf,
                        bfill[:].rearrange("p (o s) -> p o s", o=O),
                    )

            def store(st):
                orr = outs[st][:].rearrange("p (o hs) -> p o hs", o=O)
                nc.scalar.dma_start(yv[st, 0:64], orr[0:64])
                nc.sync.dma_start(yv[st, 64:128], orr[64:128])

            wins = [(st, w) for st in range(NST) for w in range(NW)]
            pending = None
            for widx, stw in enumerate(wins):
                front(*stw, widx)
                if pending is not None:
                    back(*pending)
                    if pending[1] == NW - 1:
                        store(pending[0])
                pending = stw
            back(*pending)
            store(pending[0])
    nc.compile()
    return nc


def _host_constants(weight, bias, n_discard, n_keep):
    Swin = n_discard + n_keep
    assert Swin == S and n_discard % V == 0
    first_u = n_discard // V
    DU = NU - first_u
    NPRE = DU - 1
    fill_s = first_u * V

    w = weight.reshape(O, C, n_keep).transpose(2, 1, 0).astype(np.float32)
    w_full = np.concatenate(
        [np.zeros((n_discard, C, O), np.float32), w], axis=0
    )  # (S, C, O)

    # w_intra[k=(c,vp), du*128 + v*16 + o] = w_full[u*8+vp, c, o] if vp<=v
    blk = np.stack([w_full[(first_u + du) * V:(first_u + du + 1) * V]
                    for du in range(DU)])          # (DU, V, C, O)
    tri = np.zeros((DU, C, V, V, O), np.float32)   # (du, c, vp, v, o)
    vp = np.arange(V)
    for v in range(V):
        tri[:, :, vp <= v, v, :] = blk.transpose(0, 2, 1, 3)[:, :, vp <= v]
    w_intra = tri.reshape(DU, C * V, V * O).transpose(1, 0, 2).reshape(
        P, DU * 128)

    # w_pre[k=(c,vp), pu*128 + ut*16 + o] = w_full[u*8+vp, c, o] if ut>u
    pre = np.zeros((max(NPRE, 1), C, V, NU, O), np.float32)
    for pu in range(NPRE):
        u = first_u + pu
        pre[pu, :, :, u + 1:, :] = blk[pu].transpose(1, 0, 2)[:, :, None, :]
    w_pre = pre.reshape(max(NPRE, 1), C * V, NU * O).transpose(1, 0, 2).reshape(
        P, max(NPRE, 1) * 128)

    bias32 = bias.astype(np.float32)
    consts = {
        "w_intra": np.ascontiguousarray(w_intra).astype(np.float16),
        "w_pre": np.ascontiguousarray(w_pre).astype(np.float16),
        "ident": np.eye(P, dtype=np.float16),
        "ones_k1": np.ones((1, P), np.float16),
        "biasrow": np.tile(bias32, NU)[None, :].astype(np.float16),
        "bias_fill": np.ascontiguousarray(
            np.tile(bias32[:, None], (1, max(fill_s, 1))).reshape(1, -1)
            * np.ones((P, 1), np.float32)
        ).astype(np.float16),
    }
    return consts, first_u


def _run(inputs, trace=False):
    x = np.asarray(inputs["x"], dtype=np.float32)
    weight = np.asarray(inputs["weight"], dtype=np.float32)
    bias = np.asarray(inputs["bias"], dtype=np.float32)
    n_discard = int(inputs["n_discard"])
    n_keep = int(inputs["n_keep"])
    assert x.shape == (B, C, T) and weight.shape == (O, C * n_keep)

    consts, first_u = _host_constants(weight, bias, n_discard, n_keep)
    key = ("nc", first_u)
    if key not in _cache:
        _cache[key] = _build_nc(first_u)
    nc = _cache[key]

    x16 = x.astype(np.float16)
    in_maps = []
    for b in range(B):
        m = dict(consts)
        m["x"] = np.ascontiguousarray(x16[b])
        in_maps.append(m)
    res = run_bass_kernel_spmd(nc, in_maps, list(range(B)), trace=trace)
    y = np.stack([res.results[b]["y"] for b in range(B)], axis=0)
    return y.astype(np.float32), res


def kernel(**inputs):
    y, _ = _run(inputs, trace=False)
    return y


# revision 17
# speedup vs baseline: 1.4724x; 1.1735x over previous
"""Trainium2 Bass kernel for nn_CumulativeFlattenedLinear (segment_reduce).

Per window of S=64 timesteps: per-timestep C->O projection (weights zero for
the first n_discard steps) + causal cumsum within the window, plus bias.

Strategy (data-parallel over batch, 1 batch element per core), fp16 I/O:
  - x and y cross HBM as fp16 (host converts) with 1KB contiguous runs:
    partition = 512-element time chunk, 2 supertiles. Loads/stores are split
    across the two HWDGE queues (SP + ACT) to overlap both directions.
  - Per window: 6 fp16 PE transposes build x^T (k=(c,v')) in PSUM, one ACT
    copy moves it to SBUF; 6 triangular "intra" matmuls write the window's
    (s,o)-major PSUM region; block totals accumulate into a single shared
    128-col PSUM "pre" region seeded with bias by a K=1 matmul, so prefix
    sums come out of PSUM accumulation for free.
  - One DVE broadcast-add per window evicts intra+prefix to the fp16 output
    tile; GPSIMD fills the leading n_discard positions with bias.
"""
import numpy as np

import concourse.bass as bass
import concourse.tile as tile
from concourse import bacc, mybir
from concourse.bass_utils import run_bass_kernel_spmd

F16 = mybir.dt.float16
F32 = mybir.dt.float32

B, C, T, O = 8, 16, 131072, 16
P = 128
CH = 512                  # time elems per partition per supertile (1KB fp16)
NST = T // (P * CH)       # 2 supertiles
V = 8                     # sub-block length
NU = 8                    # sub-blocks per window
S = NU * V                # 64
NW = CH // S              # windows per partition row = 8

_cache = {}

# eviction reads prefix totals straight from PSUM (2 PSUM operands on DVE);
# flip to True if hardware rejects that and route through an SBUF copy
PRE_VIA_SBUF = True


def _build_nc(first_u):
    DU = NU - first_u          # active sub-blocks (trailing)
    NPRE = DU - 1              # blocks contributing prefix totals
    fill_s = first_u * V       # s < fill_s -> output = bias

    nc = bacc.Bacc("TRN2", target_bir_lowering=False, debug=False)
    x_d = nc.dram_tensor("x", (C, T), F16, kind="ExternalInput")
    wi_d = nc.dram_tensor("w_intra", (P, DU * 128), F16, kind="ExternalInput")
    wp_d = nc.dram_tensor("w_pre", (P, max(NPRE, 1) * 128), F16,
                          kind="ExternalInput")
    ident_d = nc.dram_tensor("ident", (P, P), F16, kind="ExternalInput")
    ones_d = nc.dram_tensor("ones_k1", (1, P), F16, kind="ExternalInput")
    brow_d = nc.dram_tensor("biasrow", (1, P), F16, kind="ExternalInput")
    bfill_d = nc.dram_tensor("bias_fill", (P, max(O * NW * fill_s, 1)), F16,
                             kind="ExternalInput")
    y_d = nc.dram_tensor("y", (O, T), F16, kind="ExternalOutput")

    xv = x_d.ap().rearrange("c (st p hs) -> st p c hs", st=NST, p=P, hs=CH)
    yv = y_d.ap().rearrange("o (st p hs) -> st p o hs", st=NST, p=P, hs=CH)

    PREBASE = DU * 128         # psum tile: [intra (s>=fill_s) | pre region]

    with tile.TileContext(nc) as tc:
        with (
            tc.tile_pool(name="const", bufs=1) as cp,
            tc.tile_pool(name="io", bufs=2) as io,
            tc.tile_pool(name="shf", bufs=3) as shf,
            tc.tile_pool(name="tsb", bufs=3) as tsb,
            tc.tile_pool(name="mid", bufs=3) as mid,
            tc.tile_pool(name="psT", bufs=2, space="PSUM") as psT,
            tc.tile_pool(name="psW", bufs=2, space="PSUM") as psW,
        ):
            w_intra = cp.tile([P, DU * 128], F16, name="w_intra")
            nc.sync.dma_start(w_intra[:], wi_d.ap())
            w_pre = cp.tile([P, max(NPRE, 1) * 128], F16, name="w_pre")
            nc.sync.dma_start(w_pre[:], wp_d.ap())
            ident = cp.tile([P, P], F16, name="ident")
            nc.sync.dma_start(ident[:], ident_d.ap())
            ones = cp.tile([1, P], F16, name="ones_k1")
            nc.sync.dma_start(ones[:], ones_d.ap())
            brow = cp.tile([1, P], F16, name="biasrow")
            nc.sync.dma_start(brow[:], brow_d.ap())
            bfill = cp.tile([P, max(O * NW * fill_s, 1)], F16, name="bfill")
            nc.sync.dma_start(bfill[:], bfill_d.ap())

            # all input loads issued up-front, split across both HWDGE queues
            xins = []
            for st in range(NST):
                xin = io.tile([P, C * CH], F16, name=f"xin{st}", tag="xin")
                xr = xin[:].rearrange("p (c hs) -> p c hs", c=C)
                nc.sync.dma_start(xr[0:64], xv[st, 0:64])
                nc.scalar.dma_start(xr[64:128], xv[st, 64:128])
                xins.append(xin)
            outs = [io.tile([P, O * CH], F16, name=f"out{st}", tag="out")
                    for st in range(NST)]

            # bias fill for s < fill_s: one batched GPSIMD op per supertile,
            # issued early so it's off the per-window critical path
            if fill_s:
                for st in range(NST):
                    outf = outs[st][:].rearrange(
                        "p (o w s) -> p o w s", o=O, w=NW
                    )[:, :, :, 0:fill_s]
                    nc.gpsimd.tensor_copy(
                        outf.bitcast(F32),
                        bfill[:].rearrange(
                            "p (o w s) -> p o w s", o=O, w=NW
                        ).bitcast(F32),
                    )

            state = {}

            def front(st, w, widx):
                xin = xins[st]
                # shuffle window columns to (u, c, v) blocks; alternate the
                # engine so neither DVE nor GPSIMD becomes the bottleneck
                sh = shf.tile([P, DU * 128], F16, name="shuf", tag="shuf")
                src = xin[:].rearrange(
                    "p (c w u v) -> w p u c v", c=C, w=NW, u=NU, v=V
                )[w][:, first_u:NU]
                eng = nc.vector if widx % 2 == 0 else nc.gpsimd
                eng.tensor_copy(
                    sh[:].rearrange(
                        "p (u c v) -> p u c v", u=DU, c=C, v=V
                    ).bitcast(F32),
                    src.bitcast(F32),
                )
                pt = psT.tile([P, DU * 128], F16, name="pt", tag="pt")
                for du in range(DU):
                    nc.tensor.transpose(
                        pt[:, du * 128:(du + 1) * 128],
                        sh[:, du * 128:(du + 1) * 128],
                        ident[:],
                    )
                ts = tsb.tile([P, DU * 128], F16, name="ts", tag="ts")
                nc.scalar.copy(ts[:].bitcast(F32), pt[:].bitcast(F32))
                state[(st, w)] = ts

            def back(st, w):
                ts = state.pop((st, w))
                out_sb = outs[st]
                pw = psW.tile([P, PREBASE + P], F32, name="pw", tag="pw")
                # seed pre region with bias (sets has_written for accumulate)
                nc.tensor.matmul(
                    pw[:, PREBASE:PREBASE + P], ones[:], brow[:],
                    start=True, stop=(NPRE == 0), skip_group_check=True,
                )
                for du in range(DU):
                    lo = du * 128
                    nc.tensor.matmul(
                        pw[:, lo:lo + 128],
                        ts[:, du * 128:(du + 1) * 128],
                        w_intra[:, du * 128:(du + 1) * 128],
                        start=True, stop=True, skip_group_check=True,
                    )
                for pu in range(NPRE):
                    nc.tensor.matmul(
                        pw[:, PREBASE:PREBASE + P],
                        ts[:, pu * 128:(pu + 1) * 128],
                        w_pre[:, pu * 128:(pu + 1) * 128],
                        start=False, stop=(pu == NPRE - 1),
                        skip_group_check=True,
                    )
                if PRE_VIA_SBUF:
                    pre_sb = mid.tile([P, P], F32, name="pre_sb", tag="pre_sb")
                    nc.scalar.copy(pre_sb[:], pw[:, PREBASE:PREBASE + P])
                    pre_ap = pre_sb[:, first_u * O:]
                else:
                    pre_ap = pw[:, PREBASE + first_u * O:PREBASE + P]
                # eviction: out[(o, s)] = intra + prefix, fp16
                out4 = out_sb[:].rearrange(
                    "p (o w u v) -> w p o u v", o=O, w=NW, u=NU, v=V
                )[w][:, :, first_u:NU]
                in1 = pw[:, 0:DU * 128].rearrange(
                    "p (u v o) -> p o u v", u=DU, v=V, o=O
                )
                in2 = pre_ap.rearrange(
                    "p (u o) -> p o u", u=DU, o=O
                ).unsqueeze(3).broadcast_to([P, O, DU, V])
                nc.vector.tensor_add(out4, in1, in2)

            def store(st):
                orr = outs[st][:].rearrange("p (o hs) -> p o hs", o=O)
                nc.scalar.dma_start(yv[st, 0:64], orr[0:64])
                nc.sync.dma_start(yv[st, 64:128], orr[64:128])

            wins = [(st, w) for st in range(NST) for w in range(NW)]
            pending = None
            for widx, stw in enumerate(wins):
                front(*stw, widx)
                if pending is not None:
                    back(*pending)
                    if pending[1] == NW - 1:
                        store(pending[0])
                pending = stw
            back(*pending)
            store(pending[0])
    nc.compile()
    return nc


def _host_constants(weight, bias, n_discard, n_keep):
    Swin = n_discard + n_keep
    assert Swin == S and n_discard % V == 0
    first_u = n_discard // V
    DU = NU - first_u
    NPRE = DU - 1
    fill_s = first_u * V

    w = weight.reshape(O, C, n_keep).transpose(2, 1, 0).astype(np.float32)
    w_full = np.concatenate(
        [np.zeros((n_discard, C, O), np.float32), w], axis=0
    )  # (S, C, O)

    # w_intra[k=(c,vp), du*128 + v*16 + o] = w_full[u*8+vp, c, o] if vp<=v
    blk = np.stack([w_full[(first_u + du) * V:(first_u + du + 1) * V]
                    for du in range(DU)])          # (DU, V, C, O)
    tri = np.zeros((DU, C, V, V, O), np.float32)   # (du, c, vp, v, o)
    vp = np.arange(V)
    for v in range(V):
        tri[:, :, vp <= v, v, :] = blk.transpose(0, 2, 1, 3)[:, :, vp <= v]
    w_intra = tri.reshape(DU, C * V, V * O).transpose(1, 0, 2).reshape(
        P, DU * 128)

    # w_pre[k=(c,vp), pu*128 + ut*16 + o] = w_full[u*8+vp, c, o] if ut>u
    pre = np.zeros((max(NPRE, 1), C, V, NU, O), np.float32)
    for pu in range(NPRE):
        u = first_u + pu
        pre[pu, :, :, u + 1:, :] = blk[pu].transpose(1, 0, 2)[:, :, None, :]
    w_pre = pre.reshape(max(NPRE, 1), C * V, NU * O).transpose(1, 0, 2).reshape(
        P, max(NPRE, 1) * 128)

    bias32 = bias.astype(np.float32)
    consts = {
        "w_intra": np.ascontiguousarray(w_intra).astype(np.float16),
        "w_pre": np.ascontiguousarray(w_pre).astype(np.float16),
        "ident": np.eye(P, dtype=np.float16),
        "ones_k1": np.ones((1, P), np.float16),
        "biasrow": np.tile(bias32, NU)[None, :].astype(np.float16),
        "bias_fill": np.ascontiguousarray(
            np.tile(bias32[:, None], (1, NW * max(fill_s, 1))).reshape(1, -1)
            * np.ones((P, 1), np.float32)
        ).astype(np.float16),
    }
    return consts, first_u


def _run(inputs, trace=False):
    x = np.asarray(inputs["x"], dtype=np.float32)
    weight = np.asarray(inputs["weight"], dtype=np.float32)
    bias = np.asarray(inputs["bias"], dtype=np.float32)
    n_discard = int(inputs["n_discard"])
    n_keep = int(inputs["n_keep"])
    assert x.shape == (B, C, T) and weight.shape == (O, C * n_keep)

    consts, first_u = _host_constants(weight, bias, n_discard, n_keep)
    key = ("nc", first_u)
    if key not in _cache:
        _cache[key] = _build_nc(first_u)
    nc = _cache[key]

    x16 = x.astype(np.float16)
    in_maps = []
    for b in range(B):
        m = dict(consts)
        m["x"] = np.ascontiguousarray(x16[b])
        in_maps.append(m)
    res = run_bass_kernel_spmd(nc, in_maps, list(range(B)), trace=trace)
    y = np.stack([res.results[b]["y"] for b in range(B)], axis=0)
    return y.astype(np.float32), res


def kernel(**inputs):
    y, _ = _run(inputs, trace=False)
    return y
